# revision 15
# baseline (speedup 1.0000x reference)
"""Causal self-attention (B=4, T=2048, E=1024, H=16, D=64) on 8 trn2 cores.

Sharding: core c -> (batch b = c//2, head-group g = c%2 of 8 heads).
Each core computes qkv projection + RoPE + causal attention + its partial
output projection for its (batch, head-group); host sums the two
head-group partials per batch and transposes back.

Device data layout is feature-major ("T" suffix = [features, tokens]):
scores are computed k-major (S.T blocks [tk=128, tq]) so causal masking
skips ~half the matmuls, and softmax normalization comes from an extra
ones-column in the v operand of the PV matmul (the denominator lands in
one PSUM partition row at zero extra matmul cost).

All matmuls run in float32r (full PE rate for N>=256). The BIR verifier
requires float32r matmul operands to be *produced* as float32r, so every
matmul-feeding tile is declared float32r; engine inputs are read via
.bitcast(float32) where needed (same bits).
"""
import sys

sys.path.insert(0, "/opt/trn_rl_repo")

from contextlib import ExitStack

import numpy as np

import concourse.bass as bass
import concourse.bacc as bacc
import concourse.tile as tile
from concourse import mybir
from concourse.bass_utils import run_bass_kernel_spmd

B, T, E, H, D = 4, 2048, 1024, 16, 64
NCORES = 8
HG = H // 2          # heads per shard (8)
F = HG * D           # features per shard (512)
NPAIR = F // 128     # head pairs per shard (4)
NGRP = NPAIR // 2    # pair groups (2)
KE = E // 128        # contraction tiles over E (8)
NKT = T // 128       # k tiles (16)
TCH = 256            # t-chunk for qkv streaming
F32 = mybir.dt.float32
F32R = mybir.dt.float32r
EXP = mybir.ActivationFunctionType.Exp

_NC_CACHE = {}


def _build_program(debug_taps=False):
    key = ("nc", debug_taps)
    if key in _NC_CACHE:
        return _NC_CACHE[key]
    nc = bacc.Bacc("TRN2", target_bir_lowering=False, debug=False,
                   num_devices=NCORES)
    mm = nc.tensor.matmul
    xT = nc.dram_tensor("xT", [E, T], F32R, kind="ExternalInput").ap()
    wqT = nc.dram_tensor("wqT", [E, F], F32R, kind="ExternalInput").ap()
    wkT = nc.dram_tensor("wkT", [E, F], F32R, kind="ExternalInput").ap()
    wvT = nc.dram_tensor("wvT", [E, F], F32R, kind="ExternalInput").ap()
    wpT = nc.dram_tensor("wpT", [F, E], F32R, kind="ExternalInput").ap()
    cos2 = nc.dram_tensor("cos2", [128, T], F32, kind="ExternalInput").ap()
    ssp2 = nc.dram_tensor("ssp2", [128, T], F32, kind="ExternalInput").ap()
    perm = nc.dram_tensor("perm", [128, 128], F32R, kind="ExternalInput").ap()
    maskd = nc.dram_tensor("maskd", [128, 128], F32, kind="ExternalInput").ap()
    ones16 = nc.dram_tensor("ones16", [128, NKT], F32R, kind="ExternalInput").ap()
    bc_scratch = [nc.dram_tensor(f"bcsc{i}", [1, 1024], F32).ap()
                  for i in range(16)]
    outT = nc.dram_tensor("outT", [E, T], F32, kind="ExternalOutput").ap()
    dbg = {}
    if debug_taps:
        for nm, shp in [("dbg_q", [128, T]), ("dbg_k", [128, T]),
                        ("dbg_v3", [128, NKT * 130]), ("dbg_p", [128, 1024]),
                        ("dbg_bc", [128, 1024]), ("dbg_y", [128, T])]:
            dbg[nm] = nc.dram_tensor(nm, shp, F32, kind="ExternalOutput").ap()

    xT_r = xT.rearrange("(ke p) t -> p ke t", p=128)
    wq_r = wqT.rearrange("(ke p) j -> p ke j", p=128)
    wk_r = wkT.rearrange("(ke p) j -> p ke j", p=128)
    wv_r = wvT.rearrange("(ke p) j -> p ke j", p=128)
    wp_r = wpT.rearrange("(kf p) o -> p kf o", p=128)

    with tile.TileContext(nc) as tc:
        with ExitStack() as ctx:
            const = ctx.enter_context(tc.tile_pool(name="const", bufs=1))
            wgp = ctx.enter_context(tc.tile_pool(name="wgp", bufs=1))
            xp = ctx.enter_context(tc.tile_pool(name="xp", bufs=2))
            qkp = ctx.enter_context(tc.tile_pool(name="qkp", bufs=2))
            vp = ctx.enter_context(tc.tile_pool(name="vp", bufs=3))
            yp = ctx.enter_context(tc.tile_pool(name="yp", bufs=4))
            pp = ctx.enter_context(tc.tile_pool(name="pp", bufs=2))
            tmp = ctx.enter_context(tc.tile_pool(name="tmp", bufs=2))
            bcp = ctx.enter_context(tc.tile_pool(name="bcp", bufs=2))
            outp = ctx.enter_context(tc.tile_pool(name="outp", bufs=3))
            psA = ctx.enter_context(
                tc.tile_pool(name="psA", bufs=2, space="PSUM"))
            psS = ctx.enter_context(
                tc.tile_pool(name="psS", bufs=2, space="PSUM"))
            psY = ctx.enter_context(
                tc.tile_pool(name="psY", bufs=1, space="PSUM"))

            c_cos = const.tile([128, T], F32, tag="cos")
            c_ssp = const.tile([128, T], F32, tag="ssp")
            c_perm = const.tile([128, 128], F32R, tag="perm")
            c_mask = const.tile([128, 128], F32, tag="mask")
            c_wp = const.tile([128, NPAIR, E], F32R, tag="wp")
            nc.sync.dma_start(out=c_cos, in_=cos2)
            nc.sync.dma_start(out=c_ssp, in_=ssp2)
            nc.sync.dma_start(out=c_perm, in_=perm)
            nc.sync.dma_start(out=c_mask, in_=maskd)
            nc.sync.dma_start(out=c_wp, in_=wp_r)

            y_tiles = [None] * NPAIR

            for g in range(NGRP):
                # ---- phase A: qkv projection + rope for pairs 2g, 2g+1 ----
                wg = wgp.tile([128, KE, 768], F32R, tag="wg")
                j0 = 256 * g
                nc.sync.dma_start(out=wg[:, :, 0:256], in_=wq_r[:, :, j0:j0 + 256])
                nc.sync.dma_start(out=wg[:, :, 256:512], in_=wk_r[:, :, j0:j0 + 256])
                nc.sync.dma_start(out=wg[:, :, 512:768], in_=wv_r[:, :, j0:j0 + 256])

                pair_qk = []
                pair_v = []
                for pi in range(2):
                    qT = qkp.tile([128, T], F32R, tag="qT")
                    kT = qkp.tile([128, T], F32R, tag="kT")
                    v3 = vp.tile([128, NKT, 130], F32R, tag="v3")
                    nc.sync.dma_start(out=v3[:, :, 64], in_=ones16)
                    nc.sync.dma_start(out=v3[:, :, 129], in_=ones16)
                    pair_qk.append((qT, kT))
                    pair_v.append(v3)

                for tci in range(T // TCH):
                    tcs = tci * TCH
                    xc = xp.tile([128, KE, TCH], F32R, tag="xc")
                    nc.sync.dma_start(out=xc, in_=xT_r[:, :, tcs:tcs + TCH])
                    tcol = slice(tcs, tcs + TCH)
                    # q/k for both pairs of the group, rope fused into drain
                    for pi in range(2):
                        for sec, dst in [(0, pair_qk[pi][0]),
                                         (256, pair_qk[pi][1])]:
                            ps = psA.tile([128, TCH], F32, tag="psA")
                            wcol = sec + 128 * pi
                            for ke in range(KE):
                                mm(ps, wg[:, ke, wcol:wcol + 128],
                                   xc[:, ke, :], start=(ke == 0),
                                   stop=(ke == KE - 1), skip_group_check=True)
                            # rope: dst = ps*cos + perm @ (ps*ssp)
                            bt = tmp.tile([128, TCH], F32R, tag="bt")
                            nc.vector.tensor_mul(dst[:, tcol], ps, c_cos[:, tcol])
                            nc.vector.tensor_mul(bt, ps, c_ssp[:, tcol])
                            psw = psA.tile([128, TCH], F32, tag="psA")
                            mm(psw, c_perm, bt, start=True, stop=True,
                               skip_group_check=True)
                            nc.vector.tensor_add(
                                dst[:, tcol], dst[:, tcol].bitcast(F32), psw)
                    # v for both pairs (N=256 across the group's 256 cols)
                    for ti in range(TCH // 128):
                        tt = (tcs + ti * 128) // 128
                        psv = psA.tile([128, 256], F32, tag="psA")
                        for ke in range(KE):
                            mm(psv, xc[:, ke, 128 * ti:128 * ti + 128],
                               wg[:, ke, 512:768], start=(ke == 0),
                               stop=(ke == KE - 1), skip_group_check=True)
                        for pi in range(2):
                            nc.vector.tensor_copy(
                                pair_v[pi][:, tt, 0:64],
                                psv[:, 128 * pi:128 * pi + 64])
                            nc.vector.tensor_copy(
                                pair_v[pi][:, tt, 65:129],
                                psv[:, 128 * pi + 64:128 * pi + 128])

                if debug_taps and g == 0:
                    nc.sync.dma_start(out=dbg["dbg_q"],
                                      in_=pair_qk[0][0][:].bitcast(F32))
                    nc.sync.dma_start(out=dbg["dbg_k"],
                                      in_=pair_qk[0][1][:].bitcast(F32))
                    nc.sync.dma_start(
                        out=dbg["dbg_v3"],
                        in_=pair_v[0].rearrange("p a b -> p (a b)").bitcast(F32))
                # ---- phase B: attention for each pair of the group ----
                for pi in range(2):
                    p = 2 * g + pi
                    qT, kT = pair_qk[pi]
                    v3 = pair_v[pi]
                    yT = yp.tile([128, T], F32R, tag="yT")
                    y_tiles[p] = yT
                    for hl in range(2):  # head within pair
                        hr = 64 * hl
                        for qh in range(2):  # q half of 1024 cols
                            qbase = 1024 * qh
                            psy = psY.tile([128, 1024], F32, tag="psY")
                            kts = list(range(8 * (qh + 1)))
                            last_for = {}
                            for cs in (0, 512):
                                last_for[cs] = max(
                                    kt for kt in kts
                                    if max(qbase, 128 * kt) - qbase < cs + 512)
                            for kt in kts:
                                col_lo = max(qbase, 128 * kt) - qbase
                                pS = psS.tile([128, 1024], F32, tag="psS")
                                for cs in (0, 512):
                                    lo = max(col_lo, cs)
                                    if lo >= cs + 512:
                                        continue
                                    mm(pS[:, lo:cs + 512],
                                       kT[hr:hr + 64, 128 * kt:128 * kt + 128],
                                       qT[hr:hr + 64,
                                          qbase + lo:qbase + cs + 512],
                                       start=True, stop=True,
                                       skip_group_check=True)
                                pt = pp.tile([128, 1024], F32R, tag="pt")
                                nc.scalar.activation(
                                    pt[:, col_lo:1024], pS[:, col_lo:1024],
                                    EXP, scale=0.125)
                                if 128 * kt >= qbase:  # diagonal block in half
                                    nc.vector.tensor_mul(
                                        pt[:, col_lo:col_lo + 128],
                                        pt[:, col_lo:col_lo + 128].bitcast(F32),
                                        c_mask)
                                # per-kt v block is [v_h0|ones|v_h1|ones]:
                                # each head's operand is [v_h|ones] so y
                                # lands at psum rows 0:64, denom at row 64.
                                if (debug_taps and p == 0 and hl == 0
                                        and qh == 0 and kt == 0):
                                    nc.sync.dma_start(
                                        out=dbg["dbg_p"],
                                        in_=pt[:, 0:1024].bitcast(F32))
                                lv = v3[:, kt, 65 * hl:65 * hl + 65]
                                for cs in (0, 512):
                                    lo = max(col_lo, cs)
                                    if lo >= cs + 512:
                                        continue
                                    mm(psy[0:65, lo:cs + 512],
                                       lv, pt[:, lo:cs + 512],
                                       start=(kt == 0),
                                       stop=(kt == last_for[cs]),
                                       skip_group_check=True)
                            # divide by denominator (row 64 of psy).
                            # Broadcast via DRAM roundtrip: gpsimd
                            # partition_broadcast reads absolute partition
                            # 0 on HW, and SBUF-source step-0 DMAs are
                            # rejected; DRAM-source broadcast DMAs work.
                            bc = bcp.tile([128, 1024], F32, tag="bc")
                            nc.vector.reciprocal(bc[64:65, :],
                                                 psy[64:65, :])
                            sc = bc_scratch[(p * 2 + hl) * 2 + qh]
                            nc.sync.dma_start(out=sc, in_=bc[64:65, :])
                            nc.gpsimd.dma_start(
                                out=bc[0:64, :],
                                in_=sc.partition_broadcast(64))
                            if (debug_taps and p == 0 and hl == 0
                                    and qh == 0):
                                nc.sync.dma_start(out=dbg["dbg_bc"], in_=bc)
                            if hl == 0:
                                nc.vector.tensor_mul(
                                    yT[0:64, qbase:qbase + 1024],
                                    psy[0:64, :], bc[0:64, :])
                            else:
                                ym = tmp.tile([128, 1024], F32R, tag="ym")
                                nc.vector.tensor_mul(
                                    ym[0:64, :], psy[0:64, :], bc[0:64, :])
                                nc.sync.dma_start(
                                    out=yT[64:128, qbase:qbase + 1024],
                                    in_=ym[0:64, :])

            if debug_taps:
                nc.sync.dma_start(out=dbg["dbg_y"],
                                  in_=y_tiles[0][:].bitcast(F32))
            # ---- phase C: output projection ----
            for mo in range(E // 128):
                for cs in range(T // 512):
                    po = psA.tile([128, 512], F32, tag="psA")
                    for kp in range(NPAIR):
                        mm(po, c_wp[:, kp, 128 * mo:128 * mo + 128],
                           y_tiles[kp][:, 512 * cs:512 * cs + 512],
                           start=(kp == 0), stop=(kp == NPAIR - 1),
                           skip_group_check=True)
                    ost = outp.tile([128, 512], F32, tag="ost")
                    nc.scalar.copy(ost, po)
                    nc.sync.dma_start(
                        out=outT[128 * mo:128 * mo + 128,
                                 512 * cs:512 * cs + 512],
                        in_=ost)

    nc.compile()
    _NC_CACHE[key] = nc
    return nc


def _host_tables():
    inv_freq = 1.0 / (10000.0 ** (np.arange(0, D, 2, dtype=np.float32) / D))
    t = np.arange(T, dtype=np.float32)
    freqs = np.outer(t, inv_freq)                     # [T, 32]
    emb = np.concatenate([freqs, freqs], -1)          # [T, 64]
    cos_t = np.cos(emb).T.astype(np.float32)          # [64, T]
    sin_t = np.sin(emb).T.astype(np.float32)
    # rope(x)[d] = x[d]*cos[d] + x[d^1]*ssin[d],
    #   ssin[2i] = -sin[2i], ssin[2i+1] = +sin[2i+1]
    # device computes perm @ (x * ssp), so ssp[d] = ssin[d^1]:
    ssp = np.empty_like(sin_t)
    ssp[0::2] = sin_t[1::2]       # even d: +sin(emb[d+1])
    ssp[1::2] = -sin_t[0::2]      # odd d:  -sin(emb[d-1])
    cos2 = np.concatenate([cos_t, cos_t], 0)          # [128, T]
    ssp2 = np.concatenate([ssp, ssp], 0)
    d = np.arange(128)
    perm = (d[None, :] == (d ^ 1)[:, None]).astype(np.float32)
    r = np.arange(128)
    maskd = (r[:, None] <= r[None, :]).astype(np.float32)
    return cos2, ssp2, perm, maskd


def kernel(x, w_attn, w_proj):
    x = np.asarray(x, dtype=np.float32)
    w_attn = np.asarray(w_attn, dtype=np.float32)
    w_proj = np.asarray(w_proj, dtype=np.float32)
    cos2, ssp2, perm, maskd = _host_tables()

    nc = _build_program()
    in_maps = []
    for c in range(NCORES):
        b, g = c // 2, c % 2
        j0 = g * F
        in_maps.append({
            "xT": np.ascontiguousarray(x[b].T),
            "wqT": np.ascontiguousarray(w_attn[j0:j0 + F].T),
            "wkT": np.ascontiguousarray(w_attn[E + j0:E + j0 + F].T),
            "wvT": np.ascontiguousarray(w_attn[2 * E + j0:2 * E + j0 + F].T),
            "wpT": np.ascontiguousarray(w_proj[:, j0:j0 + F].T),
            "cos2": cos2, "ssp2": ssp2, "perm": perm, "maskd": maskd,
            "ones16": np.ones((128, 16), dtype=np.float32),
        })
    res = run_bass_kernel_spmd(nc, in_maps, core_ids=list(range(NCORES)))
    out = np.empty((B, T, E), dtype=np.float32)
    for b in range(B):
        acc = res.results[2 * b]["outT"] + res.results[2 * b + 1]["outT"]
        out[b] = acc.T
    return out


# revision 18
# speedup vs baseline: 1.0393x; 1.0393x over previous
"""Causal self-attention (B=4, T=2048, E=1024, H=16, D=64) on 8 trn2 cores.

Sharding: core c -> (batch b = c//2, head-group g = c%2 of 8 heads).
Each core computes qkv projection + RoPE + causal attention + its partial
output projection for its (batch, head-group); host sums the two
head-group partials per batch and transposes back.

Device data layout is feature-major ("T" suffix = [features, tokens]):
scores are computed k-major (S.T blocks [tk=128, tq]) so causal masking
skips ~half the matmuls, and softmax normalization comes from an extra
ones-column in the v operand of the PV matmul (the denominator lands in
one PSUM partition row at zero extra matmul cost).

All matmuls run in float32r (full PE rate for N>=256). The BIR verifier
requires float32r matmul operands to be *produced* as float32r, so every
matmul-feeding tile is declared float32r; engine inputs are read via
.bitcast(float32) where needed (same bits).
"""
import sys

sys.path.insert(0, "/opt/trn_rl_repo")

from contextlib import ExitStack

import numpy as np

import concourse.bass as bass
import concourse.bacc as bacc
import concourse.tile as tile
from concourse import mybir
from concourse.bass_utils import run_bass_kernel_spmd

B, T, E, H, D = 4, 2048, 1024, 16, 64
NCORES = 8
HG = H // 2          # heads per shard (8)
F = HG * D           # features per shard (512)
NPAIR = F // 128     # head pairs per shard (4)
NGRP = NPAIR // 2    # pair groups (2)
KE = E // 128        # contraction tiles over E (8)
NKT = T // 128       # k tiles (16)
TCH = 512            # t-chunk for qkv streaming
F32 = mybir.dt.float32
F32R = mybir.dt.float32r
EXP = mybir.ActivationFunctionType.Exp

_NC_CACHE = {}


def _build_program(debug_taps=False):
    key = ("nc", debug_taps)
    if key in _NC_CACHE:
        return _NC_CACHE[key]
    nc = bacc.Bacc("TRN2", target_bir_lowering=False, debug=False,
                   num_devices=NCORES)
    mm = nc.tensor.matmul
    xT = nc.dram_tensor("xT", [E, T], F32R, kind="ExternalInput").ap()
    wqT = nc.dram_tensor("wqT", [E, F], F32R, kind="ExternalInput").ap()
    wkT = nc.dram_tensor("wkT", [E, F], F32R, kind="ExternalInput").ap()
    wvT = nc.dram_tensor("wvT", [E, F], F32R, kind="ExternalInput").ap()
    wpT = nc.dram_tensor("wpT", [F, E], F32R, kind="ExternalInput").ap()
    cos2 = nc.dram_tensor("cos2", [128, T], F32, kind="ExternalInput").ap()
    ssp2 = nc.dram_tensor("ssp2", [128, T], F32, kind="ExternalInput").ap()
    perm = nc.dram_tensor("perm", [128, 128], F32R, kind="ExternalInput").ap()
    maskd = nc.dram_tensor("maskd", [128, 128], F32, kind="ExternalInput").ap()
    ones16 = nc.dram_tensor("ones16", [128, NKT], F32R, kind="ExternalInput").ap()
    cones = nc.dram_tensor("cones", [128, 64], F32R, kind="ExternalInput").ap()
    bc_scratch = [nc.dram_tensor(f"bcsc{i}", [1, 1024], F32).ap()
                  for i in range(16)]
    outT = nc.dram_tensor("outT", [E, T], F32, kind="ExternalOutput").ap()
    dbg = {}
    if debug_taps:
        for nm, shp in [("dbg_q", [128, T]), ("dbg_k", [128, T]),
                        ("dbg_v3", [128, NKT * 130]), ("dbg_p", [128, 1024]),
                        ("dbg_bc", [128, 1024]), ("dbg_y", [128, T])]:
            dbg[nm] = nc.dram_tensor(nm, shp, F32, kind="ExternalOutput").ap()

    xT_r = xT.rearrange("(ke p) t -> p ke t", p=128)
    wq_r = wqT.rearrange("(ke p) j -> p ke j", p=128)
    wk_r = wkT.rearrange("(ke p) j -> p ke j", p=128)
    wv_r = wvT.rearrange("(ke p) j -> p ke j", p=128)
    wp_r = wpT.rearrange("(kf p) o -> p kf o", p=128)

    with tile.TileContext(nc) as tc:
        with ExitStack() as ctx:
            const = ctx.enter_context(tc.tile_pool(name="const", bufs=1))
            wgp = ctx.enter_context(tc.tile_pool(name="wgp", bufs=1))
            xp = ctx.enter_context(tc.tile_pool(name="xp", bufs=3))
            qkp = ctx.enter_context(tc.tile_pool(name="qkp", bufs=2))
            vp = ctx.enter_context(tc.tile_pool(name="vp", bufs=2))
            yp = ctx.enter_context(tc.tile_pool(name="yp", bufs=4))
            pp = ctx.enter_context(tc.tile_pool(name="pp", bufs=2))
            tmp = ctx.enter_context(tc.tile_pool(name="tmp", bufs=2))
            bcp = ctx.enter_context(tc.tile_pool(name="bcp", bufs=2))
            outp = ctx.enter_context(tc.tile_pool(name="outp", bufs=2))
            psA = ctx.enter_context(
                tc.tile_pool(name="psA", bufs=2, space="PSUM"))
            psS = ctx.enter_context(
                tc.tile_pool(name="psS", bufs=2, space="PSUM"))
            psY = ctx.enter_context(
                tc.tile_pool(name="psY", bufs=1, space="PSUM"))

            c_cos = const.tile([128, T], F32, tag="cos")
            c_ssp = const.tile([128, T], F32, tag="ssp")
            c_perm = const.tile([128, 128], F32R, tag="perm")
            c_mask = const.tile([128, 128], F32, tag="mask")
            c_wp = const.tile([128, NPAIR, E], F32R, tag="wp")
            c_ones = const.tile([128, 64], F32R, tag="cones")
            nc.sync.dma_start(out=c_cos, in_=cos2)
            nc.sync.dma_start(out=c_ssp, in_=ssp2)
            nc.sync.dma_start(out=c_perm, in_=perm)
            nc.sync.dma_start(out=c_mask, in_=maskd)
            nc.sync.dma_start(out=c_wp, in_=wp_r)
            nc.sync.dma_start(out=c_ones, in_=cones)

            y_tiles = [None] * NPAIR

            for g in range(NGRP):
                # ---- phase A: qkv projection + rope for pairs 2g, 2g+1 ----
                wg = wgp.tile([128, KE, 768], F32R, tag="wg")
                j0 = 256 * g
                nc.sync.dma_start(out=wg[:, :, 0:256], in_=wq_r[:, :, j0:j0 + 256])
                nc.sync.dma_start(out=wg[:, :, 256:512], in_=wk_r[:, :, j0:j0 + 256])
                nc.sync.dma_start(out=wg[:, :, 512:768], in_=wv_r[:, :, j0:j0 + 256])

                pair_qk = []
                pair_v = []
                for pi in range(2):
                    qT = qkp.tile([128, T], F32R, tag="qT")
                    kT = qkp.tile([128, T], F32R, tag="kT")
                    v3 = vp.tile([128, NKT, 130], F32R, tag="v3")
                    nc.sync.dma_start(out=v3[:, :, 64], in_=ones16)
                    nc.sync.dma_start(out=v3[:, :, 129], in_=ones16)
                    pair_qk.append((qT, kT))
                    pair_v.append(v3)

                for tci in range(T // TCH):
                    tcs = tci * TCH
                    xca = xp.tile([128, KE // 2, TCH], F32R, tag="xc")
                    xcb = xp.tile([128, KE // 2, TCH], F32R, tag="xc")
                    nc.sync.dma_start(out=xca,
                                      in_=xT_r[:, 0:KE // 2, tcs:tcs + TCH])
                    nc.sync.dma_start(out=xcb,
                                      in_=xT_r[:, KE // 2:KE, tcs:tcs + TCH])

                    def xk(ke):
                        return (xca if ke < KE // 2 else xcb)[:, ke % (KE // 2), :]
                    tcol = slice(tcs, tcs + TCH)
                    # q/k for both pairs of the group, rope fused into drain
                    for pi in range(2):
                        for sec, dst in [(0, pair_qk[pi][0]),
                                         (256, pair_qk[pi][1])]:
                            ps = psA.tile([128, TCH], F32, tag="psA")
                            wcol = sec + 128 * pi
                            for ke in range(KE):
                                mm(ps, wg[:, ke, wcol:wcol + 128],
                                   xk(ke), start=(ke == 0),
                                   stop=(ke == KE - 1), skip_group_check=True)
                            # rope: dst = ps*cos + perm @ (ps*ssp)
                            bt = tmp.tile([128, TCH], F32R, tag="bt")
                            nc.vector.tensor_mul(dst[:, tcol], ps, c_cos[:, tcol])
                            nc.vector.tensor_mul(bt, ps, c_ssp[:, tcol])
                            psw = psA.tile([128, TCH], F32, tag="psA")
                            mm(psw, c_perm, bt, start=True, stop=True,
                               skip_group_check=True)
                            nc.vector.tensor_add(
                                dst[:, tcol], dst[:, tcol].bitcast(F32), psw)
                    # v for both pairs (N=256 across the group's 256 cols)
                    for ti in range(TCH // 128):
                        tt = (tcs + ti * 128) // 128
                        psv = psA.tile([128, 256], F32, tag="psA")
                        for ke in range(KE):
                            mm(psv, xk(ke)[:, 128 * ti:128 * ti + 128],
                               wg[:, ke, 512:768], start=(ke == 0),
                               stop=(ke == KE - 1), skip_group_check=True)
                        for pi in range(2):
                            nc.vector.tensor_copy(
                                pair_v[pi][:, tt, 0:64],
                                psv[:, 128 * pi:128 * pi + 64])
                            nc.vector.tensor_copy(
                                pair_v[pi][:, tt, 65:129],
                                psv[:, 128 * pi + 64:128 * pi + 128])

                if debug_taps and g == 0:
                    nc.sync.dma_start(out=dbg["dbg_q"],
                                      in_=pair_qk[0][0][:].bitcast(F32))
                    nc.sync.dma_start(out=dbg["dbg_k"],
                                      in_=pair_qk[0][1][:].bitcast(F32))
                    nc.sync.dma_start(
                        out=dbg["dbg_v3"],
                        in_=pair_v[0].rearrange("p a b -> p (a b)").bitcast(F32))
                # ---- phase B: attention for each pair of the group ----
                for pi in range(2):
                    p = 2 * g + pi
                    qT, kT = pair_qk[pi]
                    v3 = pair_v[pi]
                    yT = yp.tile([128, T], F32R, tag="yT")
                    y_tiles[p] = yT
                    for hl in range(2):  # head within pair
                        hr = 64 * hl
                        for qh in range(2):  # q half of 1024 cols
                            qbase = 1024 * qh
                            psy = psY.tile([128, 1024], F32, tag="psY")
                            kts = list(range(8 * (qh + 1)))
                            last_for = {}
                            for cs in (0, 512):
                                last_for[cs] = max(
                                    kt for kt in kts
                                    if max(qbase, 128 * kt) - qbase < cs + 512)
                            for kt in kts:
                                col_lo = max(qbase, 128 * kt) - qbase
                                pS = psS.tile([128, 1024], F32, tag="psS")
                                for cs in (0, 512):
                                    lo = max(col_lo, cs)
                                    if lo >= cs + 512:
                                        continue
                                    mm(pS[:, lo:cs + 512],
                                       kT[hr:hr + 64, 128 * kt:128 * kt + 128],
                                       qT[hr:hr + 64,
                                          qbase + lo:qbase + cs + 512],
                                       start=True, stop=True,
                                       skip_group_check=True)
                                pt = pp.tile([128, 1024], F32R, tag="pt")
                                nc.scalar.activation(
                                    pt[:, col_lo:1024], pS[:, col_lo:1024],
                                    EXP, scale=0.125)
                                if 128 * kt >= qbase:  # diagonal block in half
                                    nc.vector.tensor_mul(
                                        pt[:, col_lo:col_lo + 128],
                                        pt[:, col_lo:col_lo + 128].bitcast(F32),
                                        c_mask)
                                # per-kt v block is [v_h0|ones|v_h1|ones]:
                                # each head's operand is [v_h|ones] so y
                                # lands at psum rows 0:64, denom at row 64.
                                if (debug_taps and p == 0 and hl == 0
                                        and qh == 0 and kt == 0):
                                    nc.sync.dma_start(
                                        out=dbg["dbg_p"],
                                        in_=pt[:, 0:1024].bitcast(F32))
                                lv = v3[:, kt, 65 * hl:65 * hl + 65]
                                for cs in (0, 512):
                                    lo = max(col_lo, cs)
                                    if lo >= cs + 512:
                                        continue
                                    mm(psy[0:65, lo:cs + 512],
                                       lv, pt[:, lo:cs + 512],
                                       start=(kt == 0),
                                       stop=(kt == last_for[cs]),
                                       skip_group_check=True)
                            # divide by denominator (row 64 of psy),
                            # broadcast via DRAM roundtrip (proven).
                            bc = bcp.tile([128, 1024], F32, tag="bc")
                            nc.vector.reciprocal(bc[64:65, :],
                                                 psy[64:65, :])
                            sc = bc_scratch[(p * 2 + hl) * 2 + qh]
                            nc.sync.dma_start(out=sc, in_=bc[64:65, :])
                            nc.gpsimd.dma_start(
                                out=bc[0:64, :],
                                in_=sc.partition_broadcast(64))
                            if (debug_taps and p == 0 and hl == 0
                                    and qh == 0):
                                nc.sync.dma_start(out=dbg["dbg_bc"], in_=bc)
                            if hl == 0:
                                nc.vector.tensor_mul(
                                    yT[0:64, qbase:qbase + 1024],
                                    psy[0:64, :], bc[0:64, :])
                            else:
                                ym = tmp.tile([128, 1024], F32R, tag="ym")
                                nc.vector.tensor_mul(
                                    ym[0:64, :], psy[0:64, :], bc[0:64, :])
                                nc.sync.dma_start(
                                    out=yT[64:128, qbase:qbase + 1024],
                                    in_=ym[0:64, :])

            if debug_taps:
                nc.sync.dma_start(out=dbg["dbg_y"],
                                  in_=y_tiles[0][:].bitcast(F32))
            # ---- phase C: output projection ----
            for mo in range(E // 128):
                for cs in range(T // 512):
                    po = psA.tile([128, 512], F32, tag="psA")
                    for kp in range(NPAIR):
                        mm(po, c_wp[:, kp, 128 * mo:128 * mo + 128],
                           y_tiles[kp][:, 512 * cs:512 * cs + 512],
                           start=(kp == 0), stop=(kp == NPAIR - 1),
                           skip_group_check=True)
                    ost = outp.tile([128, 512], F32, tag="ost")
                    nc.scalar.copy(ost, po)
                    nc.sync.dma_start(
                        out=outT[128 * mo:128 * mo + 128,
                                 512 * cs:512 * cs + 512],
                        in_=ost)

    nc.compile()
    _NC_CACHE[key] = nc
    return nc


def _host_tables():
    inv_freq = 1.0 / (10000.0 ** (np.arange(0, D, 2, dtype=np.float32) / D))
    t = np.arange(T, dtype=np.float32)
    freqs = np.outer(t, inv_freq)                     # [T, 32]
    emb = np.concatenate([freqs, freqs], -1)          # [T, 64]
    cos_t = np.cos(emb).T.astype(np.float32)          # [64, T]
    sin_t = np.sin(emb).T.astype(np.float32)
    # rope(x)[d] = x[d]*cos[d] + x[d^1]*ssin[d],
    #   ssin[2i] = -sin[2i], ssin[2i+1] = +sin[2i+1]
    # device computes perm @ (x * ssp), so ssp[d] = ssin[d^1]:
    ssp = np.empty_like(sin_t)
    ssp[0::2] = sin_t[1::2]       # even d: +sin(emb[d+1])
    ssp[1::2] = -sin_t[0::2]      # odd d:  -sin(emb[d-1])
    cos2 = np.concatenate([cos_t, cos_t], 0)          # [128, T]
    ssp2 = np.concatenate([ssp, ssp], 0)
    d = np.arange(128)
    perm = (d[None, :] == (d ^ 1)[:, None]).astype(np.float32)
    r = np.arange(128)
    maskd = (r[:, None] <= r[None, :]).astype(np.float32)
    return cos2, ssp2, perm, maskd


def kernel(x, w_attn, w_proj):
    x = np.asarray(x, dtype=np.float32)
    w_attn = np.asarray(w_attn, dtype=np.float32)
    w_proj = np.asarray(w_proj, dtype=np.float32)
    cos2, ssp2, perm, maskd = _host_tables()

    nc = _build_program()
    in_maps = []
    for c in range(NCORES):
        b, g = c // 2, c % 2
        j0 = g * F
        in_maps.append({
            "xT": np.ascontiguousarray(x[b].T),
            "wqT": np.ascontiguousarray(w_attn[j0:j0 + F].T),
            "wkT": np.ascontiguousarray(w_attn[E + j0:E + j0 + F].T),
            "wvT": np.ascontiguousarray(w_attn[2 * E + j0:2 * E + j0 + F].T),
            "wpT": np.ascontiguousarray(w_proj[:, j0:j0 + F].T),
            "cos2": cos2, "ssp2": ssp2, "perm": perm, "maskd": maskd,
            "ones16": np.ones((128, 16), dtype=np.float32),
            "cones": np.ones((128, 64), dtype=np.float32),
        })
    res = run_bass_kernel_spmd(nc, in_maps, core_ids=list(range(NCORES)))
    out = np.empty((B, T, E), dtype=np.float32)
    for b in range(B):
        acc = res.results[2 * b]["outT"] + res.results[2 * b + 1]["outT"]
        out[b] = acc.T
    return out


# revision 20
# speedup vs baseline: 1.2572x; 1.2096x over previous
"""Causal self-attention (B=4, T=2048, E=1024, H=16, D=64) on 8 trn2 cores.

Sharding: core c -> (batch b = c//2, head-group g = c%2 of 8 heads).
Each core computes qkv projection + RoPE + causal attention + its partial
output projection for its (batch, head-group); host sums the two
head-group partials per batch and transposes back.

Device data layout is feature-major ("T" suffix = [features, tokens]):
scores are computed k-major (S.T blocks [tk=128, tq]) so causal masking
skips ~half the matmuls, and softmax normalization comes from an extra
ones-column in the v operand of the PV matmul (the denominator lands in
one PSUM partition row at zero extra matmul cost).

All matmuls run in float32r (full PE rate for N>=256). The BIR verifier
requires float32r matmul operands to be *produced* as float32r, so every
matmul-feeding tile is declared float32r; engine inputs are read via
.bitcast(float32) where needed (same bits).
"""
import sys

sys.path.insert(0, "/opt/trn_rl_repo")

from contextlib import ExitStack

import numpy as np

import concourse.bass as bass
import concourse.bacc as bacc
import concourse.tile as tile
from concourse import mybir
from concourse.bass_utils import run_bass_kernel_spmd

B, T, E, H, D = 4, 2048, 1024, 16, 64
NCORES = 8
HG = H // 2          # heads per shard (8)
F = HG * D           # features per shard (512)
NPAIR = F // 128     # head pairs per shard (4)
NGRP = NPAIR // 2    # pair groups (2)
KE = E // 128        # contraction tiles over E (8)
NKT = T // 128       # k tiles (16)
TCH = 512            # t-chunk for qkv streaming
F32 = mybir.dt.float32
F32R = mybir.dt.float32r
EXP = mybir.ActivationFunctionType.Exp

_NC_CACHE = {}


def _build_program(debug_taps=False):
    key = ("nc", debug_taps)
    if key in _NC_CACHE:
        return _NC_CACHE[key]
    nc = bacc.Bacc("TRN2", target_bir_lowering=False, debug=False,
                   num_devices=NCORES)
    mm = nc.tensor.matmul
    xT = nc.dram_tensor("xT", [E, T], F32R, kind="ExternalInput").ap()
    wqT = nc.dram_tensor("wqT", [E, F], F32R, kind="ExternalInput").ap()
    wkT = nc.dram_tensor("wkT", [E, F], F32R, kind="ExternalInput").ap()
    wvT = nc.dram_tensor("wvT", [E, F], F32R, kind="ExternalInput").ap()
    wpT = nc.dram_tensor("wpT", [F, E], F32R, kind="ExternalInput").ap()
    cos2 = nc.dram_tensor("cos2", [128, T], F32, kind="ExternalInput").ap()
    ssp2 = nc.dram_tensor("ssp2", [128, T], F32, kind="ExternalInput").ap()
    perm = nc.dram_tensor("perm", [128, 128], F32R, kind="ExternalInput").ap()
    maskd = nc.dram_tensor("maskd", [128, 128], F32, kind="ExternalInput").ap()
    ones16 = nc.dram_tensor("ones16", [128, NKT], F32R, kind="ExternalInput").ap()
    cones = nc.dram_tensor("cones", [128, 64], F32R, kind="ExternalInput").ap()
    bc_scratch = [nc.dram_tensor(f"bcsc{i}", [1, 512], F32).ap()
                  for i in range(32)]
    outT = nc.dram_tensor("outT", [E, T], F32, kind="ExternalOutput").ap()
    dbg = {}
    if debug_taps:
        for nm, shp in [("dbg_q", [128, T]), ("dbg_k", [128, T]),
                        ("dbg_v3", [128, NKT * 130]), ("dbg_p", [128, 1024]),
                        ("dbg_bc", [128, 1024]), ("dbg_y", [128, T])]:
            dbg[nm] = nc.dram_tensor(nm, shp, F32, kind="ExternalOutput").ap()

    xT_r = xT.rearrange("(ke p) t -> p ke t", p=128)
    wq_r = wqT.rearrange("(ke p) j -> p ke j", p=128)
    wk_r = wkT.rearrange("(ke p) j -> p ke j", p=128)
    wv_r = wvT.rearrange("(ke p) j -> p ke j", p=128)
    wp_r = wpT.rearrange("(kf p) o -> p kf o", p=128)

    with tile.TileContext(nc) as tc:
        with ExitStack() as ctx:
            const = ctx.enter_context(tc.tile_pool(name="const", bufs=1))
            wgp = ctx.enter_context(tc.tile_pool(name="wgp", bufs=1))
            xp = ctx.enter_context(tc.tile_pool(name="xp", bufs=3))
            qkp = ctx.enter_context(tc.tile_pool(name="qkp", bufs=2))
            vp = ctx.enter_context(tc.tile_pool(name="vp", bufs=2))
            yp = ctx.enter_context(tc.tile_pool(name="yp", bufs=4))
            pp = ctx.enter_context(tc.tile_pool(name="pp", bufs=3))
            tmp = ctx.enter_context(tc.tile_pool(name="tmp", bufs=2))
            bcp = ctx.enter_context(tc.tile_pool(name="bcp", bufs=2))
            outp = ctx.enter_context(tc.tile_pool(name="outp", bufs=2))
            psA = ctx.enter_context(
                tc.tile_pool(name="psA", bufs=2, space="PSUM"))
            psS = ctx.enter_context(
                tc.tile_pool(name="psS", bufs=2, space="PSUM"))
            psY = ctx.enter_context(
                tc.tile_pool(name="psY", bufs=4, space="PSUM"))

            c_cos = const.tile([128, T], F32, tag="cos")
            c_ssp = const.tile([128, T], F32, tag="ssp")
            c_perm = const.tile([128, 128], F32R, tag="perm")
            c_mask = const.tile([128, 128], F32, tag="mask")
            c_wp = const.tile([128, NPAIR, E], F32R, tag="wp")
            c_ones = const.tile([128, 64], F32R, tag="cones")
            nc.sync.dma_start(out=c_cos, in_=cos2)
            nc.sync.dma_start(out=c_ssp, in_=ssp2)
            nc.sync.dma_start(out=c_perm, in_=perm)
            nc.sync.dma_start(out=c_mask, in_=maskd)
            nc.sync.dma_start(out=c_wp, in_=wp_r)
            nc.sync.dma_start(out=c_ones, in_=cones)

            y_tiles = [None] * NPAIR

            for g in range(NGRP):
                # ---- phase A: qkv projection + rope for pairs 2g, 2g+1 ----
                wg = wgp.tile([128, KE, 768], F32R, tag="wg")
                j0 = 256 * g
                nc.sync.dma_start(out=wg[:, :, 0:256], in_=wq_r[:, :, j0:j0 + 256])
                nc.sync.dma_start(out=wg[:, :, 256:512], in_=wk_r[:, :, j0:j0 + 256])
                nc.sync.dma_start(out=wg[:, :, 512:768], in_=wv_r[:, :, j0:j0 + 256])

                pair_qk = []
                pair_v = []
                for pi in range(2):
                    qT = qkp.tile([128, T], F32R, tag="qT")
                    kT = qkp.tile([128, T], F32R, tag="kT")
                    v3 = vp.tile([128, NKT, 130], F32R, tag="v3")
                    nc.sync.dma_start(out=v3[:, :, 64], in_=ones16)
                    nc.sync.dma_start(out=v3[:, :, 129], in_=ones16)
                    pair_qk.append((qT, kT))
                    pair_v.append(v3)

                for tci in range(T // TCH):
                    tcs = tci * TCH
                    xca = xp.tile([128, KE // 2, TCH], F32R, tag="xc")
                    xcb = xp.tile([128, KE // 2, TCH], F32R, tag="xc")
                    nc.sync.dma_start(out=xca,
                                      in_=xT_r[:, 0:KE // 2, tcs:tcs + TCH])
                    nc.sync.dma_start(out=xcb,
                                      in_=xT_r[:, KE // 2:KE, tcs:tcs + TCH])

                    def xk(ke):
                        return (xca if ke < KE // 2 else xcb)[:, ke % (KE // 2), :]
                    tcol = slice(tcs, tcs + TCH)
                    # q/k for both pairs of the group, rope fused into drain
                    for pi in range(2):
                        for sec, dst in [(0, pair_qk[pi][0]),
                                         (256, pair_qk[pi][1])]:
                            ps = psA.tile([128, TCH], F32, tag="psA")
                            wcol = sec + 128 * pi
                            for ke in range(KE):
                                mm(ps, wg[:, ke, wcol:wcol + 128],
                                   xk(ke), start=(ke == 0),
                                   stop=(ke == KE - 1), skip_group_check=True)
                            # rope: dst = ps*cos + perm @ (ps*ssp)
                            bt = tmp.tile([128, TCH], F32R, tag="bt")
                            nc.vector.tensor_mul(dst[:, tcol], ps, c_cos[:, tcol])
                            nc.vector.tensor_mul(bt, ps, c_ssp[:, tcol])
                            psw = psA.tile([128, TCH], F32, tag="psA")
                            mm(psw, c_perm, bt, start=True, stop=True,
                               skip_group_check=True)
                            nc.vector.tensor_add(
                                dst[:, tcol], dst[:, tcol].bitcast(F32), psw)
                    # v for both pairs (N=256 across the group's 256 cols)
                    for ti in range(TCH // 128):
                        tt = (tcs + ti * 128) // 128
                        psv = psA.tile([128, 256], F32, tag="psA")
                        for ke in range(KE):
                            mm(psv, xk(ke)[:, 128 * ti:128 * ti + 128],
                               wg[:, ke, 512:768], start=(ke == 0),
                               stop=(ke == KE - 1), skip_group_check=True)
                        for pi in range(2):
                            nc.vector.tensor_copy(
                                pair_v[pi][:, tt, 0:64],
                                psv[:, 128 * pi:128 * pi + 64])
                            nc.vector.tensor_copy(
                                pair_v[pi][:, tt, 65:129],
                                psv[:, 128 * pi + 64:128 * pi + 128])

                if debug_taps and g == 0:
                    nc.sync.dma_start(out=dbg["dbg_q"],
                                      in_=pair_qk[0][0][:].bitcast(F32))
                    nc.sync.dma_start(out=dbg["dbg_k"],
                                      in_=pair_qk[0][1][:].bitcast(F32))
                    nc.sync.dma_start(
                        out=dbg["dbg_v3"],
                        in_=pair_v[0].rearrange("p a b -> p (a b)").bitcast(F32))
                # ---- phase B: attention, heads interleaved ----
                for pi in range(2):
                    p = 2 * g + pi
                    qT, kT = pair_qk[pi]
                    v3 = pair_v[pi]
                    yT = yp.tile([128, T], F32R, tag="yT")
                    y_tiles[p] = yT
                    for qq in range(4):  # 512-col q chunks
                        qb = 512 * qq
                        kts = list(range(4 * qq + 4))
                        last = kts[-1]
                        psy0 = psY.tile([128, 512], F32, tag="psY")
                        psy1 = psY.tile([128, 512], F32, tag="psY")
                        psy = (psy0, psy1)
                        for kt in kts:
                            col_lo = max(qb, 128 * kt) - qb
                            for hl in range(2):
                                hr = 64 * hl
                                pS = psS.tile([128, 512], F32, tag="psS")
                                mm(pS[:, col_lo:512],
                                   kT[hr:hr + 64, 128 * kt:128 * kt + 128],
                                   qT[hr:hr + 64, qb + col_lo:qb + 512],
                                   start=True, stop=True,
                                   skip_group_check=True)
                                pt = pp.tile([128, 512], F32R, tag="pt")
                                nc.scalar.activation(
                                    pt[:, col_lo:512], pS[:, col_lo:512],
                                    EXP, scale=0.125)
                                if 128 * kt >= qb:  # diagonal block
                                    nc.vector.tensor_mul(
                                        pt[:, col_lo:col_lo + 128],
                                        pt[:, col_lo:col_lo + 128]
                                        .bitcast(F32), c_mask)
                                mm(psy[hl][0:65, col_lo:512],
                                   v3[:, kt, 65 * hl:65 * hl + 65],
                                   pt[:, col_lo:512],
                                   start=(kt == 0), stop=(kt == last),
                                   skip_group_check=True)
                        for hl in range(2):
                            # divide by denominator (row 64 of psy),
                            # broadcast via DRAM roundtrip (proven).
                            bc = bcp.tile([128, 512], F32, tag="bc")
                            nc.vector.reciprocal(bc[64:65, :],
                                                 psy[hl][64:65, :])
                            sc = bc_scratch[(p * 4 + qq) * 2 + hl]
                            nc.sync.dma_start(out=sc, in_=bc[64:65, :])
                            nc.gpsimd.dma_start(
                                out=bc[0:64, :],
                                in_=sc.partition_broadcast(64))
                            if hl == 0:
                                nc.vector.tensor_mul(
                                    yT[0:64, qb:qb + 512],
                                    psy[hl][0:64, :], bc[0:64, :])
                            else:
                                ym = tmp.tile([128, 512], F32R, tag="ym")
                                nc.vector.tensor_mul(
                                    ym[0:64, :], psy[hl][0:64, :],
                                    bc[0:64, :])
                                nc.sync.dma_start(
                                    out=yT[64:128, qb:qb + 512],
                                    in_=ym[0:64, :])

            if debug_taps:
                nc.sync.dma_start(out=dbg["dbg_y"],
                                  in_=y_tiles[0][:].bitcast(F32))
            # ---- phase C: output projection ----
            for mo in range(E // 128):
                for cs in range(T // 512):
                    po = psA.tile([128, 512], F32, tag="psA")
                    for kp in range(NPAIR):
                        mm(po, c_wp[:, kp, 128 * mo:128 * mo + 128],
                           y_tiles[kp][:, 512 * cs:512 * cs + 512],
                           start=(kp == 0), stop=(kp == NPAIR - 1),
                           skip_group_check=True)
                    ost = outp.tile([128, 512], F32, tag="ost")
                    nc.scalar.copy(ost, po)
                    nc.sync.dma_start(
                        out=outT[128 * mo:128 * mo + 128,
                                 512 * cs:512 * cs + 512],
                        in_=ost)

    nc.compile()
    _NC_CACHE[key] = nc
    return nc


def _host_tables():
    inv_freq = 1.0 / (10000.0 ** (np.arange(0, D, 2, dtype=np.float32) / D))
    t = np.arange(T, dtype=np.float32)
    freqs = np.outer(t, inv_freq)                     # [T, 32]
    emb = np.concatenate([freqs, freqs], -1)          # [T, 64]
    cos_t = np.cos(emb).T.astype(np.float32)          # [64, T]
    sin_t = np.sin(emb).T.astype(np.float32)
    # rope(x)[d] = x[d]*cos[d] + x[d^1]*ssin[d],
    #   ssin[2i] = -sin[2i], ssin[2i+1] = +sin[2i+1]
    # device computes perm @ (x * ssp), so ssp[d] = ssin[d^1]:
    ssp = np.empty_like(sin_t)
    ssp[0::2] = sin_t[1::2]       # even d: +sin(emb[d+1])
    ssp[1::2] = -sin_t[0::2]      # odd d:  -sin(emb[d-1])
    cos2 = np.concatenate([cos_t, cos_t], 0)          # [128, T]
    ssp2 = np.concatenate([ssp, ssp], 0)
    d = np.arange(128)
    perm = (d[None, :] == (d ^ 1)[:, None]).astype(np.float32)
    r = np.arange(128)
    maskd = (r[:, None] <= r[None, :]).astype(np.float32)
    return cos2, ssp2, perm, maskd


def kernel(x, w_attn, w_proj):
    x = np.asarray(x, dtype=np.float32)
    w_attn = np.asarray(w_attn, dtype=np.float32)
    w_proj = np.asarray(w_proj, dtype=np.float32)
    cos2, ssp2, perm, maskd = _host_tables()

    nc = _build_program()
    in_maps = []
    for c in range(NCORES):
        b, g = c // 2, c % 2
        j0 = g * F
        in_maps.append({
            "xT": np.ascontiguousarray(x[b].T),
            "wqT": np.ascontiguousarray(w_attn[j0:j0 + F].T),
            "wkT": np.ascontiguousarray(w_attn[E + j0:E + j0 + F].T),
            "wvT": np.ascontiguousarray(w_attn[2 * E + j0:2 * E + j0 + F].T),
            "wpT": np.ascontiguousarray(w_proj[:, j0:j0 + F].T),
            "cos2": cos2, "ssp2": ssp2, "perm": perm, "maskd": maskd,
            "ones16": np.ones((128, 16), dtype=np.float32),
            "cones": np.ones((128, 64), dtype=np.float32),
        })
    res = run_bass_kernel_spmd(nc, in_maps, core_ids=list(range(NCORES)))
    out = np.empty((B, T, E), dtype=np.float32)
    for b in range(B):
        acc = res.results[2 * b]["outT"] + res.results[2 * b + 1]["outT"]
        out[b] = acc.T
    return out


# revision 21
# speedup vs baseline: 1.2651x; 1.0063x over previous
"""Causal self-attention (B=4, T=2048, E=1024, H=16, D=64) on 8 trn2 cores.

Sharding: core c -> (batch b = c//2, head-group g = c%2 of 8 heads).
Each core computes qkv projection + RoPE + causal attention + its partial
output projection for its (batch, head-group); host sums the two
head-group partials per batch and transposes back.

Device data layout is feature-major ("T" suffix = [features, tokens]):
scores are computed k-major (S.T blocks [tk=128, tq]) so causal masking
skips ~half the matmuls, and softmax normalization comes from an extra
ones-column in the v operand of the PV matmul (the denominator lands in
one PSUM partition row at zero extra matmul cost).

All matmuls run in float32r (full PE rate for N>=256). The BIR verifier
requires float32r matmul operands to be *produced* as float32r, so every
matmul-feeding tile is declared float32r; engine inputs are read via
.bitcast(float32) where needed (same bits).
"""
import sys

sys.path.insert(0, "/opt/trn_rl_repo")

from contextlib import ExitStack

import numpy as np

import concourse.bass as bass
import concourse.bacc as bacc
import concourse.tile as tile
from concourse import mybir
from concourse.bass_utils import run_bass_kernel_spmd

B, T, E, H, D = 4, 2048, 1024, 16, 64
NCORES = 8
HG = H // 2          # heads per shard (8)
F = HG * D           # features per shard (512)
NPAIR = F // 128     # head pairs per shard (4)
NGRP = NPAIR // 2    # pair groups (2)
KE = E // 128        # contraction tiles over E (8)
NKT = T // 128       # k tiles (16)
TCH = 512            # t-chunk for qkv streaming
F32 = mybir.dt.float32
F32R = mybir.dt.float32r
EXP = mybir.ActivationFunctionType.Exp

_NC_CACHE = {}


def _build_program(debug_taps=False):
    key = ("nc", debug_taps)
    if key in _NC_CACHE:
        return _NC_CACHE[key]
    nc = bacc.Bacc("TRN2", target_bir_lowering=False, debug=False,
                   num_devices=NCORES)
    mm = nc.tensor.matmul
    xT = nc.dram_tensor("xT", [E, T], F32R, kind="ExternalInput").ap()
    wqT = nc.dram_tensor("wqT", [E, F], F32R, kind="ExternalInput").ap()
    wkT = nc.dram_tensor("wkT", [E, F], F32R, kind="ExternalInput").ap()
    wvT = nc.dram_tensor("wvT", [E, F], F32R, kind="ExternalInput").ap()
    wpT = nc.dram_tensor("wpT", [F, E], F32R, kind="ExternalInput").ap()
    cos2 = nc.dram_tensor("cos2", [128, T], F32, kind="ExternalInput").ap()
    ssp2 = nc.dram_tensor("ssp2", [128, T], F32, kind="ExternalInput").ap()
    perm = nc.dram_tensor("perm", [128, 128], F32R, kind="ExternalInput").ap()
    maskd = nc.dram_tensor("maskd", [128, 128], F32, kind="ExternalInput").ap()
    ones16 = nc.dram_tensor("ones16", [128, NKT], F32R, kind="ExternalInput").ap()
    cones = nc.dram_tensor("cones", [128, 64], F32R, kind="ExternalInput").ap()
    bc_scratch = [nc.dram_tensor(f"bcsc{i}", [1, 512], F32).ap()
                  for i in range(32)]
    outT = nc.dram_tensor("outT", [E, T], F32, kind="ExternalOutput").ap()
    dbg = {}
    if debug_taps:
        for nm, shp in [("dbg_q", [128, T]), ("dbg_k", [128, T]),
                        ("dbg_v3", [128, NKT * 130]), ("dbg_p", [128, 1024]),
                        ("dbg_bc", [128, 1024]), ("dbg_y", [128, T])]:
            dbg[nm] = nc.dram_tensor(nm, shp, F32, kind="ExternalOutput").ap()

    xT_r = xT.rearrange("(ke p) t -> p ke t", p=128)
    wq_r = wqT.rearrange("(ke p) j -> p ke j", p=128)
    wk_r = wkT.rearrange("(ke p) j -> p ke j", p=128)
    wv_r = wvT.rearrange("(ke p) j -> p ke j", p=128)
    wp_r = wpT.rearrange("(kf p) o -> p kf o", p=128)

    with tile.TileContext(nc) as tc:
        with ExitStack() as ctx:
            const = ctx.enter_context(tc.tile_pool(name="const", bufs=1))
            wgp = ctx.enter_context(tc.tile_pool(name="wgp", bufs=1))
            xp = ctx.enter_context(tc.tile_pool(name="xp", bufs=3))
            qkp = ctx.enter_context(tc.tile_pool(name="qkp", bufs=2))
            vp = ctx.enter_context(tc.tile_pool(name="vp", bufs=2))
            yp = ctx.enter_context(tc.tile_pool(name="yp", bufs=4))
            pp = ctx.enter_context(tc.tile_pool(name="pp", bufs=4))
            tmp = ctx.enter_context(tc.tile_pool(name="tmp", bufs=2))
            bcp = ctx.enter_context(tc.tile_pool(name="bcp", bufs=2))
            outp = ctx.enter_context(tc.tile_pool(name="outp", bufs=2))
            psA = ctx.enter_context(
                tc.tile_pool(name="psA", bufs=2, space="PSUM"))
            psS = ctx.enter_context(
                tc.tile_pool(name="psS", bufs=3, space="PSUM"))
            psY = ctx.enter_context(
                tc.tile_pool(name="psY", bufs=3, space="PSUM"))

            c_cos = const.tile([128, T], F32, tag="cos")
            c_ssp = const.tile([128, T], F32, tag="ssp")
            c_perm = const.tile([128, 128], F32R, tag="perm")
            c_mask = const.tile([128, 128], F32, tag="mask")
            c_wp = const.tile([128, NPAIR, E], F32R, tag="wp")
            c_ones = const.tile([128, 64], F32R, tag="cones")
            nc.sync.dma_start(out=c_cos, in_=cos2)
            nc.sync.dma_start(out=c_ssp, in_=ssp2)
            nc.sync.dma_start(out=c_perm, in_=perm)
            nc.sync.dma_start(out=c_mask, in_=maskd)
            nc.sync.dma_start(out=c_wp, in_=wp_r)
            nc.sync.dma_start(out=c_ones, in_=cones)

            y_tiles = [None] * NPAIR

            for g in range(NGRP):
                # ---- phase A: qkv projection + rope for pairs 2g, 2g+1 ----
                wg = wgp.tile([128, KE, 768], F32R, tag="wg")
                j0 = 256 * g
                nc.sync.dma_start(out=wg[:, :, 0:256], in_=wq_r[:, :, j0:j0 + 256])
                nc.sync.dma_start(out=wg[:, :, 256:512], in_=wk_r[:, :, j0:j0 + 256])
                nc.sync.dma_start(out=wg[:, :, 512:768], in_=wv_r[:, :, j0:j0 + 256])

                pair_qk = []
                pair_v = []
                for pi in range(2):
                    qT = qkp.tile([128, T], F32R, tag="qT")
                    kT = qkp.tile([128, T], F32R, tag="kT")
                    v3 = vp.tile([128, NKT, 130], F32R, tag="v3")
                    nc.sync.dma_start(out=v3[:, :, 64], in_=ones16)
                    nc.sync.dma_start(out=v3[:, :, 129], in_=ones16)
                    pair_qk.append((qT, kT))
                    pair_v.append(v3)

                for tci in range(T // TCH):
                    tcs = tci * TCH
                    xca = xp.tile([128, KE // 2, TCH], F32R, tag="xc")
                    xcb = xp.tile([128, KE // 2, TCH], F32R, tag="xc")
                    nc.sync.dma_start(out=xca,
                                      in_=xT_r[:, 0:KE // 2, tcs:tcs + TCH])
                    nc.sync.dma_start(out=xcb,
                                      in_=xT_r[:, KE // 2:KE, tcs:tcs + TCH])

                    def xk(ke):
                        return (xca if ke < KE // 2 else xcb)[:, ke % (KE // 2), :]
                    tcol = slice(tcs, tcs + TCH)
                    # q/k for both pairs of the group, rope fused into drain
                    for pi in range(2):
                        for sec, dst in [(0, pair_qk[pi][0]),
                                         (256, pair_qk[pi][1])]:
                            ps = psA.tile([128, TCH], F32, tag="psA")
                            wcol = sec + 128 * pi
                            for ke in range(KE):
                                mm(ps, wg[:, ke, wcol:wcol + 128],
                                   xk(ke), start=(ke == 0),
                                   stop=(ke == KE - 1), skip_group_check=True)
                            # rope: dst = ps*cos + perm @ (ps*ssp)
                            bt = tmp.tile([128, TCH], F32R, tag="bt")
                            nc.vector.tensor_mul(dst[:, tcol], ps, c_cos[:, tcol])
                            nc.vector.tensor_mul(bt, ps, c_ssp[:, tcol])
                            psw = psA.tile([128, TCH], F32, tag="psA")
                            mm(psw, c_perm, bt, start=True, stop=True,
                               skip_group_check=True)
                            nc.vector.tensor_add(
                                dst[:, tcol], dst[:, tcol].bitcast(F32), psw)
                    # v for both pairs (N=256 across the group's 256 cols)
                    for ti in range(TCH // 128):
                        tt = (tcs + ti * 128) // 128
                        psv = psA.tile([128, 256], F32, tag="psA")
                        for ke in range(KE):
                            mm(psv, xk(ke)[:, 128 * ti:128 * ti + 128],
                               wg[:, ke, 512:768], start=(ke == 0),
                               stop=(ke == KE - 1), skip_group_check=True)
                        for pi in range(2):
                            nc.vector.tensor_copy(
                                pair_v[pi][:, tt, 0:64],
                                psv[:, 128 * pi:128 * pi + 64])
                            nc.vector.tensor_copy(
                                pair_v[pi][:, tt, 65:129],
                                psv[:, 128 * pi + 64:128 * pi + 128])

                if debug_taps and g == 0:
                    nc.sync.dma_start(out=dbg["dbg_q"],
                                      in_=pair_qk[0][0][:].bitcast(F32))
                    nc.sync.dma_start(out=dbg["dbg_k"],
                                      in_=pair_qk[0][1][:].bitcast(F32))
                    nc.sync.dma_start(
                        out=dbg["dbg_v3"],
                        in_=pair_v[0].rearrange("p a b -> p (a b)").bitcast(F32))
                # ---- phase B: attention, heads interleaved ----
                for pi in range(2):
                    p = 2 * g + pi
                    qT, kT = pair_qk[pi]
                    v3 = pair_v[pi]
                    yT = yp.tile([128, T], F32R, tag="yT")
                    y_tiles[p] = yT
                    for qq in range(4):  # 512-col q chunks
                        qb = 512 * qq
                        kts = list(range(4 * qq + 4))
                        last = kts[-1]
                        psy0 = psY.tile([128, 512], F32, tag="psY")
                        psy1 = psY.tile([128, 512], F32, tag="psY")
                        psy = (psy0, psy1)
                        for kt in kts:
                            col_lo = max(qb, 128 * kt) - qb
                            for hl in range(2):
                                hr = 64 * hl
                                pS = psS.tile([128, 512], F32, tag="psS")
                                mm(pS[:, col_lo:512],
                                   kT[hr:hr + 64, 128 * kt:128 * kt + 128],
                                   qT[hr:hr + 64, qb + col_lo:qb + 512],
                                   start=True, stop=True,
                                   skip_group_check=True)
                                pt = pp.tile([128, 512], F32R, tag="pt")
                                nc.scalar.activation(
                                    pt[:, col_lo:512], pS[:, col_lo:512],
                                    EXP, scale=0.125)
                                if 128 * kt >= qb:  # diagonal block
                                    nc.vector.tensor_mul(
                                        pt[:, col_lo:col_lo + 128],
                                        pt[:, col_lo:col_lo + 128]
                                        .bitcast(F32), c_mask)
                                mm(psy[hl][0:65, col_lo:512],
                                   v3[:, kt, 65 * hl:65 * hl + 65],
                                   pt[:, col_lo:512],
                                   start=(kt == 0), stop=(kt == last),
                                   skip_group_check=True)
                        for hl in range(2):
                            # divide by denominator (row 64 of psy),
                            # broadcast via DRAM roundtrip (proven).
                            bc = bcp.tile([128, 512], F32, tag="bc")
                            nc.vector.reciprocal(bc[64:65, :],
                                                 psy[hl][64:65, :])
                            sc = bc_scratch[(p * 4 + qq) * 2 + hl]
                            nc.sync.dma_start(out=sc, in_=bc[64:65, :])
                            nc.gpsimd.dma_start(
                                out=bc[0:64, :],
                                in_=sc.partition_broadcast(64))
                            if hl == 0:
                                nc.vector.tensor_mul(
                                    yT[0:64, qb:qb + 512],
                                    psy[hl][0:64, :], bc[0:64, :])
                            else:
                                ym = tmp.tile([128, 512], F32R, tag="ym")
                                nc.vector.tensor_mul(
                                    ym[0:64, :], psy[hl][0:64, :],
                                    bc[0:64, :])
                                nc.sync.dma_start(
                                    out=yT[64:128, qb:qb + 512],
                                    in_=ym[0:64, :])

            if debug_taps:
                nc.sync.dma_start(out=dbg["dbg_y"],
                                  in_=y_tiles[0][:].bitcast(F32))
            # ---- phase C: output projection ----
            for mo in range(E // 128):
                for cs in range(T // 512):
                    po = psA.tile([128, 512], F32, tag="psA")
                    for kp in range(NPAIR):
                        mm(po, c_wp[:, kp, 128 * mo:128 * mo + 128],
                           y_tiles[kp][:, 512 * cs:512 * cs + 512],
                           start=(kp == 0), stop=(kp == NPAIR - 1),
                           skip_group_check=True)
                    ost = outp.tile([128, 512], F32, tag="ost")
                    nc.scalar.copy(ost, po)
                    nc.sync.dma_start(
                        out=outT[128 * mo:128 * mo + 128,
                                 512 * cs:512 * cs + 512],
                        in_=ost)

    nc.compile()
    _NC_CACHE[key] = nc
    return nc


def _host_tables():
    inv_freq = 1.0 / (10000.0 ** (np.arange(0, D, 2, dtype=np.float32) / D))
    t = np.arange(T, dtype=np.float32)
    freqs = np.outer(t, inv_freq)                     # [T, 32]
    emb = np.concatenate([freqs, freqs], -1)          # [T, 64]
    cos_t = np.cos(emb).T.astype(np.float32)          # [64, T]
    sin_t = np.sin(emb).T.astype(np.float32)
    # rope(x)[d] = x[d]*cos[d] + x[d^1]*ssin[d],
    #   ssin[2i] = -sin[2i], ssin[2i+1] = +sin[2i+1]
    # device computes perm @ (x * ssp), so ssp[d] = ssin[d^1]:
    ssp = np.empty_like(sin_t)
    ssp[0::2] = sin_t[1::2]       # even d: +sin(emb[d+1])
    ssp[1::2] = -sin_t[0::2]      # odd d:  -sin(emb[d-1])
    cos2 = np.concatenate([cos_t, cos_t], 0)          # [128, T]
    ssp2 = np.concatenate([ssp, ssp], 0)
    d = np.arange(128)
    perm = (d[None, :] == (d ^ 1)[:, None]).astype(np.float32)
    r = np.arange(128)
    maskd = (r[:, None] <= r[None, :]).astype(np.float32)
    return cos2, ssp2, perm, maskd


def kernel(x, w_attn, w_proj):
    x = np.asarray(x, dtype=np.float32)
    w_attn = np.asarray(w_attn, dtype=np.float32)
    w_proj = np.asarray(w_proj, dtype=np.float32)
    cos2, ssp2, perm, maskd = _host_tables()

    nc = _build_program()
    in_maps = []
    for c in range(NCORES):
        b, g = c // 2, c % 2
        j0 = g * F
        in_maps.append({
            "xT": np.ascontiguousarray(x[b].T),
            "wqT": np.ascontiguousarray(w_attn[j0:j0 + F].T),
            "wkT": np.ascontiguousarray(w_attn[E + j0:E + j0 + F].T),
            "wvT": np.ascontiguousarray(w_attn[2 * E + j0:2 * E + j0 + F].T),
            "wpT": np.ascontiguousarray(w_proj[:, j0:j0 + F].T),
            "cos2": cos2, "ssp2": ssp2, "perm": perm, "maskd": maskd,
            "ones16": np.ones((128, 16), dtype=np.float32),
            "cones": np.ones((128, 64), dtype=np.float32),
        })
    res = run_bass_kernel_spmd(nc, in_maps, core_ids=list(range(NCORES)))
    out = np.empty((B, T, E), dtype=np.float32)
    for b in range(B):
        acc = res.results[2 * b]["outT"] + res.results[2 * b + 1]["outT"]
        out[b] = acc.T
    return out


# revision 22
# speedup vs baseline: 1.4088x; 1.1136x over previous
"""Causal self-attention (B=4, T=2048, E=1024, H=16, D=64) on 8 trn2 cores.

Sharding: core c -> (batch b = c//2, head-group g = c%2 of 8 heads).
Each core computes qkv projection + RoPE + causal attention + its partial
output projection for its (batch, head-group); host sums the two
head-group partials per batch and transposes back.

Device data layout is feature-major ("T" suffix = [features, tokens]):
scores are computed k-major (S.T blocks [tk=128, tq]) so causal masking
skips ~half the matmuls, and softmax normalization comes from an extra
ones-column in the v operand of the PV matmul (the denominator lands in
one PSUM partition row at zero extra matmul cost).

All matmuls run in float32r (full PE rate for N>=256). The BIR verifier
requires float32r matmul operands to be *produced* as float32r, so every
matmul-feeding tile is declared float32r; engine inputs are read via
.bitcast(float32) where needed (same bits).
"""
import sys

sys.path.insert(0, "/opt/trn_rl_repo")

from contextlib import ExitStack

import numpy as np

import concourse.bass as bass
import concourse.bacc as bacc
import concourse.tile as tile
from concourse import mybir
from concourse.bass_utils import run_bass_kernel_spmd

B, T, E, H, D = 4, 2048, 1024, 16, 64
NCORES = 8
HG = H // 2          # heads per shard (8)
F = HG * D           # features per shard (512)
NPAIR = F // 128     # head pairs per shard (4)
NGRP = NPAIR // 2    # pair groups (2)
KE = E // 128        # contraction tiles over E (8)
NKT = T // 128       # k tiles (16)
TCH = 512            # t-chunk for qkv streaming
F32 = mybir.dt.float32
F32R = mybir.dt.float32r
EXP = mybir.ActivationFunctionType.Exp

_NC_CACHE = {}


def _build_program(debug_taps=False):
    key = ("nc", debug_taps)
    if key in _NC_CACHE:
        return _NC_CACHE[key]
    nc = bacc.Bacc("TRN2", target_bir_lowering=False, debug=False,
                   num_devices=NCORES)
    mm = nc.tensor.matmul
    xT = nc.dram_tensor("xT", [E, T], F32R, kind="ExternalInput").ap()
    wqT = nc.dram_tensor("wqT", [E, F], F32R, kind="ExternalInput").ap()
    wkT = nc.dram_tensor("wkT", [E, F], F32R, kind="ExternalInput").ap()
    wvT = nc.dram_tensor("wvT", [E, F], F32R, kind="ExternalInput").ap()
    wpT = nc.dram_tensor("wpT", [F, E], F32R, kind="ExternalInput").ap()
    cos2 = nc.dram_tensor("cos2", [128, T], F32, kind="ExternalInput").ap()
    ssp2 = nc.dram_tensor("ssp2", [128, T], F32, kind="ExternalInput").ap()
    perm = nc.dram_tensor("perm", [128, 128], F32R, kind="ExternalInput").ap()
    maskd = nc.dram_tensor("maskd", [128, 128], F32, kind="ExternalInput").ap()
    ones16 = nc.dram_tensor("ones16", [128, NKT], F32R, kind="ExternalInput").ap()
    cones = nc.dram_tensor("cones", [128, 64], F32R, kind="ExternalInput").ap()
    bc_scratch = [nc.dram_tensor(f"bcsc{i}", [1, 512], F32).ap()
                  for i in range(32)]
    outT = nc.dram_tensor("outT", [E, T], F32, kind="ExternalOutput").ap()
    dbg = {}
    if debug_taps:
        for nm, shp in [("dbg_q", [128, T]), ("dbg_k", [128, T]),
                        ("dbg_v3", [128, NKT * 130]), ("dbg_p", [128, 1024]),
                        ("dbg_bc", [128, 1024]), ("dbg_y", [128, T])]:
            dbg[nm] = nc.dram_tensor(nm, shp, F32, kind="ExternalOutput").ap()

    xT_r = xT.rearrange("(ke p) t -> p ke t", p=128)
    wq_r = wqT.rearrange("(ke p) j -> p ke j", p=128)
    wk_r = wkT.rearrange("(ke p) j -> p ke j", p=128)
    wv_r = wvT.rearrange("(ke p) j -> p ke j", p=128)
    wp_r = wpT.rearrange("(kf p) o -> p kf o", p=128)

    with tile.TileContext(nc) as tc:
        with ExitStack() as ctx:
            const = ctx.enter_context(tc.tile_pool(name="const", bufs=1))
            wgp = ctx.enter_context(tc.tile_pool(name="wgp", bufs=1))
            xp = ctx.enter_context(tc.tile_pool(name="xp", bufs=3))
            qkp = ctx.enter_context(tc.tile_pool(name="qkp", bufs=2))
            vp = ctx.enter_context(tc.tile_pool(name="vp", bufs=2))
            yp = ctx.enter_context(tc.tile_pool(name="yp", bufs=4))
            pp = ctx.enter_context(tc.tile_pool(name="pp", bufs=4))
            tmp = ctx.enter_context(tc.tile_pool(name="tmp", bufs=2))
            bcp = ctx.enter_context(tc.tile_pool(name="bcp", bufs=2))
            outp = ctx.enter_context(tc.tile_pool(name="outp", bufs=2))
            psA = ctx.enter_context(
                tc.tile_pool(name="psA", bufs=2, space="PSUM"))
            psS = ctx.enter_context(
                tc.tile_pool(name="psS", bufs=2, space="PSUM"))
            psY = ctx.enter_context(
                tc.tile_pool(name="psY", bufs=2, space="PSUM"))

            c_cos = const.tile([128, T], F32, tag="cos")
            c_ssp = const.tile([128, T], F32, tag="ssp")
            c_perm = const.tile([128, 128], F32R, tag="perm")
            c_mask = const.tile([128, 128], F32, tag="mask")
            c_wp = const.tile([128, NPAIR, E], F32R, tag="wp")
            c_ones = const.tile([128, 64], F32R, tag="cones")
            nc.sync.dma_start(out=c_cos, in_=cos2)
            nc.sync.dma_start(out=c_ssp, in_=ssp2)
            nc.sync.dma_start(out=c_perm, in_=perm)
            nc.sync.dma_start(out=c_mask, in_=maskd)
            nc.sync.dma_start(out=c_wp, in_=wp_r)
            nc.sync.dma_start(out=c_ones, in_=cones)

            y_tiles = [None] * NPAIR

            for g in range(NGRP):
                # ---- phase A: qkv projection + rope for pairs 2g, 2g+1 ----
                wg = wgp.tile([128, KE, 768], F32R, tag="wg")
                j0 = 256 * g
                nc.sync.dma_start(out=wg[:, :, 0:256], in_=wq_r[:, :, j0:j0 + 256])
                nc.sync.dma_start(out=wg[:, :, 256:512], in_=wk_r[:, :, j0:j0 + 256])
                nc.sync.dma_start(out=wg[:, :, 512:768], in_=wv_r[:, :, j0:j0 + 256])

                pair_qk = []
                pair_v = []
                for pi in range(2):
                    qT = qkp.tile([128, T], F32R, tag="qT")
                    kT = qkp.tile([128, T], F32R, tag="kT")
                    v3 = vp.tile([128, NKT, 130], F32R, tag="v3")
                    nc.sync.dma_start(out=v3[:, :, 64], in_=ones16)
                    nc.sync.dma_start(out=v3[:, :, 129], in_=ones16)
                    pair_qk.append((qT, kT))
                    pair_v.append(v3)

                for tci in range(T // TCH):
                    tcs = tci * TCH
                    xca = xp.tile([128, KE // 2, TCH], F32R, tag="xc")
                    xcb = xp.tile([128, KE // 2, TCH], F32R, tag="xc")
                    nc.sync.dma_start(out=xca,
                                      in_=xT_r[:, 0:KE // 2, tcs:tcs + TCH])
                    nc.sync.dma_start(out=xcb,
                                      in_=xT_r[:, KE // 2:KE, tcs:tcs + TCH])

                    def xk(ke):
                        return (xca if ke < KE // 2 else xcb)[:, ke % (KE // 2), :]
                    tcol = slice(tcs, tcs + TCH)
                    # q/k for both pairs of the group, rope fused into drain
                    for pi in range(2):
                        for sec, dst in [(0, pair_qk[pi][0]),
                                         (256, pair_qk[pi][1])]:
                            ps = psA.tile([128, TCH], F32, tag="psA")
                            wcol = sec + 128 * pi
                            for ke in range(KE):
                                mm(ps, wg[:, ke, wcol:wcol + 128],
                                   xk(ke), start=(ke == 0),
                                   stop=(ke == KE - 1), skip_group_check=True)
                            # rope: dst = ps*cos + perm @ (ps*ssp)
                            bt = tmp.tile([128, TCH], F32R, tag="bt")
                            nc.vector.tensor_mul(dst[:, tcol], ps, c_cos[:, tcol])
                            nc.vector.tensor_mul(bt, ps, c_ssp[:, tcol])
                            psw = psA.tile([128, TCH], F32, tag="psA")
                            mm(psw, c_perm, bt, start=True, stop=True,
                               skip_group_check=True)
                            nc.vector.tensor_add(
                                dst[:, tcol], dst[:, tcol].bitcast(F32), psw)
                    # v for both pairs (N=256 across the group's 256 cols)
                    for ti in range(TCH // 128):
                        tt = (tcs + ti * 128) // 128
                        psv = psA.tile([128, 256], F32, tag="psA")
                        for ke in range(KE):
                            mm(psv, xk(ke)[:, 128 * ti:128 * ti + 128],
                               wg[:, ke, 512:768], start=(ke == 0),
                               stop=(ke == KE - 1), skip_group_check=True)
                        for pi in range(2):
                            nc.scalar.copy(
                                pair_v[pi][:, tt, 0:64],
                                psv[:, 128 * pi:128 * pi + 64])
                            nc.scalar.copy(
                                pair_v[pi][:, tt, 65:129],
                                psv[:, 128 * pi + 64:128 * pi + 128])

                if debug_taps and g == 0:
                    nc.sync.dma_start(out=dbg["dbg_q"],
                                      in_=pair_qk[0][0][:].bitcast(F32))
                    nc.sync.dma_start(out=dbg["dbg_k"],
                                      in_=pair_qk[0][1][:].bitcast(F32))
                    nc.sync.dma_start(
                        out=dbg["dbg_v3"],
                        in_=pair_v[0].rearrange("p a b -> p (a b)").bitcast(F32))
                # ---- phase B: attention, heads interleaved ----
                for pi in range(2):
                    p = 2 * g + pi
                    qT, kT = pair_qk[pi]
                    v3 = pair_v[pi]
                    yT = yp.tile([128, T], F32R, tag="yT")
                    y_tiles[p] = yT
                    for qq in range(4):  # 512-col q chunks
                        qb = 512 * qq
                        kts = list(range(4 * qq + 4))
                        last = kts[-1]
                        psy0 = psY.tile([128, 512], F32, tag="psY")
                        psy1 = psY.tile([128, 512], F32, tag="psY")
                        psy = (psy0, psy1)
                        for kt in kts:
                            col_lo = max(qb, 128 * kt) - qb
                            w = 512 - col_lo
                            pS = psS.tile([128, 1024], F32, tag="psS")
                            for hl in range(2):
                                hr = 64 * hl
                                mm(pS[:, 512 * hl + col_lo:512 * hl + 512],
                                   kT[hr:hr + 64, 128 * kt:128 * kt + 128],
                                   qT[hr:hr + 64, qb + col_lo:qb + 512],
                                   start=True, stop=True,
                                   skip_group_check=True)
                            pt = pp.tile([128, 1024], F32R, tag="pt")
                            pS2 = pS.rearrange("p (h c) -> p h c", h=2)
                            pt2 = pt.rearrange("p (h c) -> p h c", h=2)
                            nc.scalar.activation(
                                pt2[:, :, col_lo:512], pS2[:, :, col_lo:512],
                                EXP, scale=0.125)
                            if 128 * kt >= qb:  # diagonal block, both heads
                                for hl in range(2):
                                    o = 512 * hl + col_lo
                                    nc.gpsimd.tensor_mul(
                                        pt[:, o:o + 128],
                                        pt[:, o:o + 128].bitcast(F32),
                                        c_mask)
                            for hl in range(2):
                                mm(psy[hl][0:65, col_lo:512],
                                   v3[:, kt, 65 * hl:65 * hl + 65],
                                   pt[:, 512 * hl + col_lo:512 * hl + 512],
                                   start=(kt == 0), stop=(kt == last),
                                   skip_group_check=True)
                        for hl in range(2):
                            # divide by denominator (row 64 of psy),
                            # broadcast via DRAM roundtrip (proven).
                            bc = bcp.tile([128, 512], F32, tag="bc")
                            nc.vector.reciprocal(bc[64:65, :],
                                                 psy[hl][64:65, :])
                            sc = bc_scratch[(p * 4 + qq) * 2 + hl]
                            nc.sync.dma_start(out=sc, in_=bc[64:65, :])
                            nc.gpsimd.dma_start(
                                out=bc[0:64, :],
                                in_=sc.partition_broadcast(64))
                            if hl == 0:
                                nc.vector.tensor_mul(
                                    yT[0:64, qb:qb + 512],
                                    psy[hl][0:64, :], bc[0:64, :])
                            else:
                                ym = tmp.tile([128, 512], F32R, tag="ym")
                                nc.vector.tensor_mul(
                                    ym[0:64, :], psy[hl][0:64, :],
                                    bc[0:64, :])
                                nc.sync.dma_start(
                                    out=yT[64:128, qb:qb + 512],
                                    in_=ym[0:64, :])

            if debug_taps:
                nc.sync.dma_start(out=dbg["dbg_y"],
                                  in_=y_tiles[0][:].bitcast(F32))
            # ---- phase C: output projection ----
            for mo in range(E // 128):
                for cs in range(T // 512):
                    po = psA.tile([128, 512], F32, tag="psA")
                    for kp in range(NPAIR):
                        mm(po, c_wp[:, kp, 128 * mo:128 * mo + 128],
                           y_tiles[kp][:, 512 * cs:512 * cs + 512],
                           start=(kp == 0), stop=(kp == NPAIR - 1),
                           skip_group_check=True)
                    ost = outp.tile([128, 512], F32, tag="ost")
                    nc.scalar.copy(ost, po)
                    nc.sync.dma_start(
                        out=outT[128 * mo:128 * mo + 128,
                                 512 * cs:512 * cs + 512],
                        in_=ost)

    nc.compile()
    _NC_CACHE[key] = nc
    return nc


def _host_tables():
    inv_freq = 1.0 / (10000.0 ** (np.arange(0, D, 2, dtype=np.float32) / D))
    t = np.arange(T, dtype=np.float32)
    freqs = np.outer(t, inv_freq)                     # [T, 32]
    emb = np.concatenate([freqs, freqs], -1)          # [T, 64]
    cos_t = np.cos(emb).T.astype(np.float32)          # [64, T]
    sin_t = np.sin(emb).T.astype(np.float32)
    # rope(x)[d] = x[d]*cos[d] + x[d^1]*ssin[d],
    #   ssin[2i] = -sin[2i], ssin[2i+1] = +sin[2i+1]
    # device computes perm @ (x * ssp), so ssp[d] = ssin[d^1]:
    ssp = np.empty_like(sin_t)
    ssp[0::2] = sin_t[1::2]       # even d: +sin(emb[d+1])
    ssp[1::2] = -sin_t[0::2]      # odd d:  -sin(emb[d-1])
    cos2 = np.concatenate([cos_t, cos_t], 0)          # [128, T]
    ssp2 = np.concatenate([ssp, ssp], 0)
    d = np.arange(128)
    perm = (d[None, :] == (d ^ 1)[:, None]).astype(np.float32)
    r = np.arange(128)
    maskd = (r[:, None] <= r[None, :]).astype(np.float32)
    return cos2, ssp2, perm, maskd


def kernel(x, w_attn, w_proj):
    x = np.asarray(x, dtype=np.float32)
    w_attn = np.asarray(w_attn, dtype=np.float32)
    w_proj = np.asarray(w_proj, dtype=np.float32)
    cos2, ssp2, perm, maskd = _host_tables()

    nc = _build_program()
    in_maps = []
    for c in range(NCORES):
        b, g = c // 2, c % 2
        j0 = g * F
        in_maps.append({
            "xT": np.ascontiguousarray(x[b].T),
            "wqT": np.ascontiguousarray(w_attn[j0:j0 + F].T),
            "wkT": np.ascontiguousarray(w_attn[E + j0:E + j0 + F].T),
            "wvT": np.ascontiguousarray(w_attn[2 * E + j0:2 * E + j0 + F].T),
            "wpT": np.ascontiguousarray(w_proj[:, j0:j0 + F].T),
            "cos2": cos2, "ssp2": ssp2, "perm": perm, "maskd": maskd,
            "ones16": np.ones((128, 16), dtype=np.float32),
            "cones": np.ones((128, 64), dtype=np.float32),
        })
    res = run_bass_kernel_spmd(nc, in_maps, core_ids=list(range(NCORES)))
    out = np.empty((B, T, E), dtype=np.float32)
    for b in range(B):
        acc = res.results[2 * b]["outT"] + res.results[2 * b + 1]["outT"]
        out[b] = acc.T
    return out
